# revision 7
# baseline (speedup 1.0000x reference)
"""Trainium2 Bass kernel for nn_CellDecoder (span-pool + ffnn + biaffine pairs).

Strategy: head_idx/tail_idx only reference E=256 entities, so instead of
computing the biaffine per pair (P=65536), each core builds the full E x E
biaffine logit table for its batch (small matmuls, fp32r) and the per-pair
work becomes a pure table lookup done with the GPSIMD ap_gather custom op.

Sharding: cores 0-3 handle batch 0, cores 4-7 batch 1. Each core replicates
its batch's table build and gathers its quarter of that batch's pairs
(bucketed host-side by e1%128//16 onto the 8 GPSIMD cores).

Perf notes:
- All matmul operands are float32r (TF32-like, 1 cyc/row at n>=256 vs 4 for
  fp32); inputs are declared float32r in DRAM so plain HWDGE DMAs feed the
  PE without a cast pass and the GPSIMD queue stays empty.
- The ap_gather ucode library is loaded explicitly at kernel start so the
  ModifyPoolConfig overlaps the weight stream instead of serializing before
  the gather (it costs ~60us when auto-inserted late).
- Big tensors are host-packed to dense [128, cols] so every DMA descriptor
  is a large contiguous run.
- DMA issue order matches compute order: pooling operands, head-ffnn
  weights, biaffine weights, tail-ffnn weights.
"""

import os

os.environ.setdefault("JAX_PLATFORMS", "axon,cpu")

import numpy as np
import einops

import concourse.bass as bass
import concourse.tile as tile
from concourse import bacc, mybir, library_config
from concourse.bass_utils import run_bass_kernel_spmd

dt = mybir.dt

B, T, D, E, P = 2, 512, 768, 256, 65536
MLP = 2 * D  # 1536
H1, H2 = MLP // 2, MLP // 4  # 768, 384
NL = 5
OUT = 2
N_CORES = 8

KT_MLP = MLP // 128  # 12
KT_H1 = H1 // 128  # 6
KT_H2 = H2 // 128  # 3
KT_T = T // 128  # 4
MT_D = D // 128  # 6
MT_H1 = H1 // 128  # 6
MT_H2 = H2 // 128  # 3
MT_E = E // 128  # 2

_cache: dict = {}


def _build(ni: int):
    """Build + compile the SPMD program; ni = padded per-gpsimd-core index count."""
    if ni in _cache:
        return _cache[ni]

    nc = bacc.Bacc("TRN2", target_bir_lowering=False, debug=False, num_devices=N_CORES)

    f32, f32r, i16 = dt.float32, dt.float32r, dt.int16

    # [128, cols] host-packed operand tensors (f32r bits == f32 bits)
    d_hs = nc.dram_tensor("hs", [128, KT_T * D], f32r, kind="ExternalInput")
    d_maskn = nc.dram_tensor("masknT", [128, KT_T * E], f32r, kind="ExternalInput")
    d_ohlab = nc.dram_tensor("ohlab", [NL, E], f32r, kind="ExternalInput")
    d_embw = nc.dram_tensor("embw", [NL, D], f32r, kind="ExternalInput")
    d_wh1 = nc.dram_tensor("Wh1", [128, KT_MLP * H1], f32r, kind="ExternalInput")
    d_wt1 = nc.dram_tensor("Wt1", [128, KT_MLP * H1], f32r, kind="ExternalInput")
    d_wh2 = nc.dram_tensor("Wh2", [128, KT_H1 * H2], f32r, kind="ExternalInput")
    d_wt2 = nc.dram_tensor("Wt2", [128, KT_H1 * H2], f32r, kind="ExternalInput")
    d_bh1 = nc.dram_tensor("bh1t", [128, MT_H1], f32, kind="ExternalInput")
    d_bt1 = nc.dram_tensor("bt1t", [128, MT_H1], f32, kind="ExternalInput")
    d_bh2 = nc.dram_tensor("bh2t", [128, MT_H2], f32, kind="ExternalInput")
    d_bt2 = nc.dram_tensor("bt2t", [128, MT_H2], f32, kind="ExternalInput")
    d_wb0 = nc.dram_tensor("Wbil0", [128, KT_H2 * H2], f32r, kind="ExternalInput")
    d_wb1 = nc.dram_tensor("Wbil1", [128, KT_H2 * H2], f32r, kind="ExternalInput")
    d_wlin = nc.dram_tensor("Wlin", [128, 2 * KT_H2 * OUT], f32r, kind="ExternalInput")
    d_blin = nc.dram_tensor("blin", [1, OUT], f32, kind="ExternalInput")
    d_ones = nc.dram_tensor("ones", [1, E], f32r, kind="ExternalInput")
    d_idx = nc.dram_tensor("idx", [128, ni // 16], i16, kind="ExternalInput")
    d_gout = nc.dram_tensor("gout", [128, ni, OUT], f32, kind="ExternalOutput")

    with tile.TileContext(nc) as tc:
        with (
            tc.tile_pool(name="wbig", bufs=1) as wbig,
            tc.tile_pool(name="wsml", bufs=1) as wsml,
            tc.tile_pool(name="act", bufs=1) as act,
            tc.tile_pool(name="ps", bufs=4, space="PSUM") as ps,
            tc.tile_pool(name="ps1", bufs=2, space="PSUM") as ps1,
        ):
            # ap_gather ucode load up front, overlapping the DMA stream.
            # The first APGather execution pays a ~61us one-time Q7-side
            # init, so fire a zero-dependency dummy gather immediately —
            # it runs concurrently with the whole matmul phase and the
            # real gather then dispatches in ~1us.
            nc.gpsimd.load_library(library_config.ap_gather)
            dumtab = wsml.tile([128, 1, OUT], f32, tag="dumtab")
            dumidx = wsml.tile([128, 1], i16, tag="dumidx")
            dumout = wsml.tile([128, 16, OUT], f32, tag="dumout")
            nc.gpsimd.memset(dumtab[:], 0.0)
            nc.gpsimd.memset(dumidx[:], 0)
            nc.gpsimd.ap_gather(
                dumout[:], dumtab[:], dumidx[:], channels=128, num_elems=1,
                d=OUT, num_idxs=16,
            )

            def load(pool, name, dram, shape, dtype=f32r, engine=None):
                t = pool.tile(shape, dtype, tag=name, name=name)
                src = dram.ap()
                if len(shape) == 3:
                    src = src.rearrange("p (kt n) -> p kt n", kt=shape[1])
                (engine or nc.sync).dma_start(t[:], src)
                return t

            # smalls on the scalar HWDGE ring so the sync ring is all bulk
            idx = load(wsml, "idx", d_idx, [128, ni // 16], i16, nc.scalar)
            blin = load(wsml, "blin", d_blin, [1, OUT], f32, nc.scalar)
            ones = load(wsml, "ones", d_ones, [1, E], f32r, nc.scalar)
            b1 = {
                "h": load(wsml, "b1h", d_bh1, [128, MT_H1], f32, nc.scalar),
                "t": load(wsml, "b1t", d_bt1, [128, MT_H1], f32, nc.scalar),
            }
            b2 = {
                "h": load(wsml, "b2h", d_bh2, [128, MT_H2], f32, nc.scalar),
                "t": load(wsml, "b2t", d_bt2, [128, MT_H2], f32, nc.scalar),
            }
            ohlab = load(wsml, "ohlab", d_ohlab, [NL, E], f32r, nc.scalar)
            embw = load(wsml, "embw", d_embw, [NL, D], f32r, nc.scalar)

            # bulk stream in compute order
            hs = load(wbig, "hs", d_hs, [128, KT_T, D])
            maskn = load(wsml, "maskn", d_maskn, [128, KT_T, E])
            w1 = {"h": load(wbig, "w1h", d_wh1, [128, KT_MLP, H1])}
            w2 = {"h": load(wbig, "w2h", d_wh2, [128, KT_H1, H2])}
            wb = [
                load(wsml, "wb0", d_wb0, [128, KT_H2, H2]),
                load(wsml, "wb1", d_wb1, [128, KT_H2, H2]),
            ]
            wlin = load(wsml, "wlin", d_wlin, [128, 2 * KT_H2, OUT])
            w1["t"] = load(wbig, "w1t", d_wt1, [128, KT_MLP, H1])
            w2["t"] = load(wbig, "w2t", d_wt2, [128, KT_H1, H2])

            # ---- ent_repr^T = [pooled^T ; emb^T]  [128, 12, E] (f32r) ----
            entT = act.tile([128, KT_MLP, E], f32r, tag="entT")
            for mt in range(MT_D):
                p = ps.tile([128, E], f32, tag="mm")
                for kt in range(KT_T):
                    nc.tensor.matmul(
                        p[:],
                        hs[:, kt, mt * 128 : (mt + 1) * 128],
                        maskn[:, kt, :],
                        start=(kt == 0),
                        stop=(kt == KT_T - 1),
                    )
                nc.vector.tensor_copy(entT[:, mt, :], p[:])
            for mt in range(MT_D):
                p = ps.tile([128, E], f32, tag="mm")
                nc.tensor.matmul(
                    p[:],
                    embw[:, mt * 128 : (mt + 1) * 128],
                    ohlab[:],
                    start=True,
                    stop=True,
                )
                nc.vector.tensor_copy(entT[:, MT_D + mt, :], p[:])

            # ---- ffnn chains; head first so tail weights can still stream ----
            h2T = {}

            def ffnn(side):
                h1T = act.tile(
                    [128, KT_H1, E], f32r, tag=f"h1T{side}", name=f"h1T{side}"
                )
                for mt in range(MT_H1):
                    p = ps.tile([128, E], f32, tag="mm")
                    for kt in range(KT_MLP):
                        nc.tensor.matmul(
                            p[:],
                            w1[side][:, kt, mt * 128 : (mt + 1) * 128],
                            entT[:, kt, :],
                            start=(kt == 0),
                            stop=(kt == KT_MLP - 1),
                        )
                    nc.scalar.activation(
                        h1T[:, mt, :],
                        p[:],
                        mybir.ActivationFunctionType.Relu,
                        bias=b1[side][:, mt : mt + 1],
                    )
                h2T[side] = act.tile(
                    [128, KT_H2, E], f32r, tag=f"h2T{side}", name=f"h2T{side}"
                )
                for mt in range(MT_H2):
                    p = ps.tile([128, E], f32, tag="mm")
                    for kt in range(KT_H1):
                        nc.tensor.matmul(
                            p[:],
                            w2[side][:, kt, mt * 128 : (mt + 1) * 128],
                            h1T[:, kt, :],
                            start=(kt == 0),
                            stop=(kt == KT_H1 - 1),
                        )
                    nc.scalar.activation(
                        h2T[side][:, mt, :],
                        p[:],
                        mybir.ActivationFunctionType.Relu,
                        bias=b2[side][:, mt : mt + 1],
                    )

            ffnn("h")

            # ---- N_o^T and lin_h (depend only on the head chain) ----
            nT = []
            for o in range(OUT):
                nTo = act.tile([128, KT_H2, E], f32r, tag=f"nT{o}", name=f"nT{o}")
                for mt in range(MT_H2):
                    p = ps.tile([128, E], f32, tag="mm")
                    for kt in range(KT_H2):
                        nc.tensor.matmul(
                            p[:],
                            wb[o][:, kt, mt * 128 : (mt + 1) * 128],
                            h2T["h"][:, kt, :],
                            start=(kt == 0),
                            stop=(kt == KT_H2 - 1),
                        )
                    nc.vector.tensor_copy(nTo[:, mt, :], p[:])
                nT.append(nTo)

            linh = []
            for o in range(OUT):
                lh = act.tile([1, E], f32r, tag=f"linh{o}", name=f"linh{o}")
                p = ps1.tile([1, E], f32, tag="lin")
                for kt in range(KT_H2):
                    nc.tensor.matmul(
                        p[:],
                        wlin[:, kt, o : o + 1],
                        h2T["h"][:, kt, :],
                        start=(kt == 0),
                        stop=(kt == KT_H2 - 1),
                    )
                nc.vector.tensor_copy(lh[:], p[:])
                linh.append(lh)

            ffnn("t")

            lint = []
            for o in range(OUT):
                lt = act.tile([1, E], f32r, tag=f"lint{o}", name=f"lint{o}")
                p = ps1.tile([1, E], f32, tag="lin")
                for kt in range(KT_H2):
                    nc.tensor.matmul(
                        p[:],
                        wlin[:, KT_H2 + kt, o : o + 1],
                        h2T["t"][:, kt, :],
                        start=(kt == 0),
                        stop=(kt == KT_H2 - 1),
                    )
                # + b_lin[o] folded in via bias
                nc.scalar.activation(
                    lt[:],
                    p[:],
                    mybir.ActivationFunctionType.Identity,
                    bias=blin[:, o : o + 1],
                )
                lint.append(lt)

            # ---- table slab [128, 2*E, OUT]: partition p holds e1=p rows
            #      (elems 0:256) and e1=128+p rows (elems 256:512) ----
            slab = act.tile([128, 2 * E, OUT], f32, tag="slab")
            for o in range(OUT):
                for mt in range(MT_E):
                    p = ps.tile([128, E], f32, tag="mm")
                    for kt in range(KT_H2):
                        nc.tensor.matmul(
                            p[:],
                            nT[o][:, kt, mt * 128 : (mt + 1) * 128],
                            h2T["t"][:, kt, :],
                            start=(kt == 0),
                            stop=False,
                        )
                    nc.tensor.matmul(
                        p[:],
                        linh[o][:, mt * 128 : (mt + 1) * 128],
                        ones[:],
                        start=False,
                        stop=False,
                    )
                    nc.tensor.matmul(
                        p[:],
                        ones[:, 0:128],
                        lint[o][:],
                        start=False,
                        stop=True,
                    )
                    nc.vector.tensor_copy(slab[:, mt * E : (mt + 1) * E, o], p[:])

            # ---- gather + output ----
            gout = act.tile([128, ni, OUT], f32, tag="gout")
            nc.gpsimd.ap_gather(
                gout[:], slab[:], idx[:], channels=128, num_elems=2 * E, d=OUT,
                num_idxs=ni,
            )
            nc.sync.dma_start(d_gout.ap(), gout[:])

    nc.compile()
    _cache[ni] = nc
    return nc


def _pack(w, kt):
    """[kt*128, n] row-major -> [128, kt*n] partition-packed."""
    n = w.shape[1]
    return np.ascontiguousarray(
        w.reshape(kt, 128, n).transpose(1, 0, 2).reshape(128, kt * n)
    )


def _prep_host(inputs):
    """Host-side index preprocessing -> per-core in_maps + assembly info."""
    hs = np.asarray(inputs["hidden_states"], dtype=np.float32)
    start = np.asarray(inputs["entity_start"]).astype(np.int64)
    end = np.asarray(inputs["entity_end"]).astype(np.int64)
    label = np.asarray(inputs["entity_label"]).astype(np.int64)
    head_idx = np.asarray(inputs["head_idx"]).astype(np.int64)
    tail_idx = np.asarray(inputs["tail_idx"]).astype(np.int64)

    t = np.arange(T)
    mask = (
        (t[None, None, :] >= start[:, :, None]) & (t[None, None, :] < end[:, :, None])
    ).astype(np.float32)  # [B,E,T]
    counts = np.maximum(mask.sum(-1, keepdims=True), 1.0)
    masknT = (mask / counts).transpose(0, 2, 1)  # [B,T,E]

    ohlab = np.zeros((B, NL, E), np.float32)
    for b in range(B):
        ohlab[b, label[b], np.arange(E)] = 1.0

    def f32(x):
        return np.ascontiguousarray(np.asarray(x, dtype=np.float32))

    w_bil = f32(inputs["W_bil"])
    shared = {
        "embw": f32(inputs["entity_emb_w"]),
        "Wh1": _pack(f32(inputs["Wh1"]), KT_MLP),
        "Wt1": _pack(f32(inputs["Wt1"]), KT_MLP),
        "Wh2": _pack(f32(inputs["Wh2"]), KT_H1),
        "Wt2": _pack(f32(inputs["Wt2"]), KT_H1),
        "Wbil0": _pack(w_bil[0], KT_H2),
        "Wbil1": _pack(w_bil[1], KT_H2),
        "Wlin": _pack(f32(inputs["W_lin"]), 2 * KT_H2),
        "blin": f32(inputs["b_lin"]).reshape(1, OUT),
        "ones": np.ones((1, E), np.float32),
        "bh1t": np.ascontiguousarray(f32(inputs["bh1"]).reshape(MT_H1, 128).T),
        "bt1t": np.ascontiguousarray(f32(inputs["bt1"]).reshape(MT_H1, 128).T),
        "bh2t": np.ascontiguousarray(f32(inputs["bh2"]).reshape(MT_H2, 128).T),
        "bt2t": np.ascontiguousarray(f32(inputs["bt2"]).reshape(MT_H2, 128).T),
    }

    # --- pair bucketing per core ---
    q = P // 4  # pairs per core
    cores = []
    ni_needed = 0
    for i in range(N_CORES):
        b, quarter = divmod(i, 4)
        sl = slice(quarter * q, (quarter + 1) * q)
        e1 = head_idx[b, sl]
        e2 = tail_idx[b, sl]
        part = e1 % 128  # target partition (= gpsimd channel)
        gcore = part // 16  # gpsimd core 0..7
        elem = e2 + 256 * (e1 // 128)  # index into per-partition table row pair
        order = np.argsort(gcore, kind="stable")
        counts_g = np.bincount(gcore, minlength=8)
        ni_needed = max(ni_needed, int(counts_g.max()))
        cores.append((b, sl, part, order, counts_g, elem))

    ni = -(-ni_needed // 16) * 16  # round up to multiple of 16

    in_maps = []
    assembly = []
    for i in range(N_CORES):
        b, sl, part, order, counts_g, elem = cores[i]
        elem_sorted = elem[order]
        gcore_sorted = (part // 16)[order]
        starts = np.zeros(8, np.int64)
        starts[1:] = np.cumsum(counts_g)[:-1]
        slot = np.arange(len(order)) - starts[gcore_sorted]  # slot within bucket
        idx_arr = np.zeros((128, ni // 16), np.int16)
        for j in range(8):
            lj = elem_sorted[gcore_sorted == j].astype(np.int16)
            pad = np.zeros(ni, np.int16)
            pad[: len(lj)] = lj
            idx_arr[16 * j : 16 * (j + 1)] = einops.rearrange(pad, "(s p) -> p s", p=16)
        m = dict(shared)
        m["hs"] = _pack(hs[b], KT_T)
        m["masknT"] = _pack(masknT[b], KT_T)
        m["ohlab"] = np.ascontiguousarray(ohlab[b])
        m["idx"] = idx_arr
        in_maps.append(m)
        # assembly: out[b, sl][order] = gout[part_sorted, slot, :]
        assembly.append((b, sl, part[order], slot, order))

    return in_maps, assembly, ni


def kernel(**inputs) -> np.ndarray:
    in_maps, assembly, ni = _prep_host(inputs)
    nc = _build(ni)
    res = run_bass_kernel_spmd(nc, in_maps, list(range(N_CORES)))
    out = np.zeros((B, P, OUT), np.float32)
    for i in range(N_CORES):
        b, sl, part_sorted, slot, order = assembly[i]
        gathered = res.results[i]["gout"][part_sorted, slot, :]  # [q, OUT]
        block = np.empty_like(gathered)
        block[order] = gathered
        out[b, sl] = block
    return out


# revision 8
# speedup vs baseline: 1.0236x; 1.0236x over previous
"""Trainium2 Bass kernel for nn_CellDecoder (span-pool + ffnn + biaffine pairs).

Strategy: head_idx/tail_idx only reference E=256 entities, so instead of
computing the biaffine per pair (P=65536), each core builds the full E x E
biaffine logit table for its batch (small matmuls, fp32r) and the per-pair
work becomes a pure table lookup done with the GPSIMD ap_gather custom op.

Sharding: cores 0-3 handle batch 0, cores 4-7 batch 1. Each core replicates
its batch's table build and gathers its quarter of that batch's pairs
(bucketed host-side by e1%128//16 onto the 8 GPSIMD cores).

Perf notes:
- All matmul operands are float32r (TF32-like, 1 cyc/row at n>=256 vs 4 for
  fp32); inputs are declared float32r in DRAM so plain HWDGE DMAs feed the
  PE without a cast pass and the GPSIMD queue stays empty.
- The ap_gather ucode library is loaded explicitly at kernel start so the
  ModifyPoolConfig overlaps the weight stream instead of serializing before
  the gather (it costs ~60us when auto-inserted late).
- Big tensors are host-packed to dense [128, cols] so every DMA descriptor
  is a large contiguous run.
- DMA issue order matches compute order: pooling operands, head-ffnn
  weights, biaffine weights, tail-ffnn weights.
"""

import os

os.environ.setdefault("JAX_PLATFORMS", "axon,cpu")

import numpy as np
import einops

import concourse.bass as bass
import concourse.tile as tile
from concourse import bacc, mybir, library_config
from concourse.bass_utils import run_bass_kernel_spmd

dt = mybir.dt

B, T, D, E, P = 2, 512, 768, 256, 65536
MLP = 2 * D  # 1536
H1, H2 = MLP // 2, MLP // 4  # 768, 384
NL = 5
OUT = 2
N_CORES = 8

KT_MLP = MLP // 128  # 12
KT_H1 = H1 // 128  # 6
KT_H2 = H2 // 128  # 3
KT_T = T // 128  # 4
MT_D = D // 128  # 6
MT_H1 = H1 // 128  # 6
MT_H2 = H2 // 128  # 3
MT_E = E // 128  # 2

_cache: dict = {}


def _build(ni: int):
    """Build + compile the SPMD program; ni = padded per-gpsimd-core index count."""
    if ni in _cache:
        return _cache[ni]

    nc = bacc.Bacc("TRN2", target_bir_lowering=False, debug=False, num_devices=N_CORES)

    f32, f32r, i16 = dt.float32, dt.float32r, dt.int16

    # [128, cols] host-packed operand tensors (f32r bits == f32 bits)
    d_hs = nc.dram_tensor("hs", [128, KT_T * D], f32r, kind="ExternalInput")
    d_maskn = nc.dram_tensor("masknT", [128, KT_T * E], f32r, kind="ExternalInput")
    d_ohlab = nc.dram_tensor("ohlab", [NL, E], f32r, kind="ExternalInput")
    d_embw = nc.dram_tensor("embw", [NL, D], f32r, kind="ExternalInput")
    d_wh1 = nc.dram_tensor("Wh1", [128, KT_MLP * H1], f32r, kind="ExternalInput")
    d_wt1 = nc.dram_tensor("Wt1", [128, KT_MLP * H1], f32r, kind="ExternalInput")
    d_wh2 = nc.dram_tensor("Wh2", [128, KT_H1 * H2], f32r, kind="ExternalInput")
    d_wt2 = nc.dram_tensor("Wt2", [128, KT_H1 * H2], f32r, kind="ExternalInput")
    d_bh1 = nc.dram_tensor("bh1t", [128, MT_H1], f32, kind="ExternalInput")
    d_bt1 = nc.dram_tensor("bt1t", [128, MT_H1], f32, kind="ExternalInput")
    d_bh2 = nc.dram_tensor("bh2t", [128, MT_H2], f32, kind="ExternalInput")
    d_bt2 = nc.dram_tensor("bt2t", [128, MT_H2], f32, kind="ExternalInput")
    d_wb0 = nc.dram_tensor("Wbil0", [128, KT_H2 * H2], f32r, kind="ExternalInput")
    d_wb1 = nc.dram_tensor("Wbil1", [128, KT_H2 * H2], f32r, kind="ExternalInput")
    d_wlin = nc.dram_tensor("Wlin", [128, 2 * KT_H2 * OUT], f32r, kind="ExternalInput")
    d_blin = nc.dram_tensor("blin", [1, OUT], f32, kind="ExternalInput")
    d_ones = nc.dram_tensor("ones", [1, E], f32r, kind="ExternalInput")
    d_idx = nc.dram_tensor("idx", [128, ni // 16], i16, kind="ExternalInput")
    d_gout = nc.dram_tensor("gout", [128, ni, OUT], f32, kind="ExternalOutput")

    with tile.TileContext(nc) as tc:
        with (
            tc.tile_pool(name="wbig", bufs=1) as wbig,
            tc.tile_pool(name="wsml", bufs=1) as wsml,
            tc.tile_pool(name="act", bufs=1) as act,
            tc.tile_pool(name="ps", bufs=4, space="PSUM") as ps,
            tc.tile_pool(name="ps1", bufs=2, space="PSUM") as ps1,
        ):
            # ap_gather ucode load up front, overlapping the DMA stream.
            # The first APGather execution pays a ~61us one-time Q7-side
            # init, so fire a zero-dependency dummy gather immediately —
            # it runs concurrently with the whole matmul phase and the
            # real gather then dispatches in ~1us.
            nc.gpsimd.load_library(library_config.ap_gather)
            dumtab = wsml.tile([128, 1, OUT], f32, tag="dumtab")
            dumidx = wsml.tile([128, 1], i16, tag="dumidx")
            dumout = wsml.tile([128, 16, OUT], f32, tag="dumout")
            nc.gpsimd.memset(dumtab[:], 0.0)
            nc.gpsimd.memset(dumidx[:], 0)
            nc.gpsimd.ap_gather(
                dumout[:], dumtab[:], dumidx[:], channels=128, num_elems=1,
                d=OUT, num_idxs=16,
            )

            def load(pool, name, dram, shape, dtype=f32r, engine=None):
                t = pool.tile(shape, dtype, tag=name, name=name)
                src = dram.ap()
                if len(shape) == 3:
                    src = src.rearrange("p (kt n) -> p kt n", kt=shape[1])
                (engine or nc.sync).dma_start(t[:], src)
                return t

            # smalls on the scalar HWDGE ring so the sync ring is all bulk
            idx = load(wsml, "idx", d_idx, [128, ni // 16], i16, nc.scalar)
            blin = load(wsml, "blin", d_blin, [1, OUT], f32, nc.scalar)
            ones = load(wsml, "ones", d_ones, [1, E], f32r, nc.scalar)
            b1 = {
                "h": load(wsml, "b1h", d_bh1, [128, MT_H1], f32, nc.scalar),
                "t": load(wsml, "b1t", d_bt1, [128, MT_H1], f32, nc.scalar),
            }
            b2 = {
                "h": load(wsml, "b2h", d_bh2, [128, MT_H2], f32, nc.scalar),
                "t": load(wsml, "b2t", d_bt2, [128, MT_H2], f32, nc.scalar),
            }
            ohlab = load(wsml, "ohlab", d_ohlab, [NL, E], f32r, nc.scalar)
            embw = load(wsml, "embw", d_embw, [NL, D], f32r, nc.scalar)

            # bulk stream in compute order
            hs = load(wbig, "hs", d_hs, [128, KT_T, D])
            maskn = load(wsml, "maskn", d_maskn, [128, KT_T, E])
            w1 = {"h": load(wbig, "w1h", d_wh1, [128, KT_MLP, H1])}
            w2 = {"h": load(wbig, "w2h", d_wh2, [128, KT_H1, H2])}
            wb = [
                load(wsml, "wb0", d_wb0, [128, KT_H2, H2]),
                load(wsml, "wb1", d_wb1, [128, KT_H2, H2]),
            ]
            wlin = load(wsml, "wlin", d_wlin, [128, 2 * KT_H2, OUT])
            w1["t"] = load(wbig, "w1t", d_wt1, [128, KT_MLP, H1])
            w2["t"] = load(wbig, "w2t", d_wt2, [128, KT_H1, H2])

            # ---- ent_repr^T = [pooled^T ; emb^T]  [128, 12, E] (f32r) ----
            entT = act.tile([128, KT_MLP, E], f32r, tag="entT")
            for mt in range(MT_D):
                p = ps.tile([128, E], f32, tag="mm")
                for kt in range(KT_T):
                    nc.tensor.matmul(
                        p[:],
                        hs[:, kt, mt * 128 : (mt + 1) * 128],
                        maskn[:, kt, :],
                        start=(kt == 0),
                        stop=(kt == KT_T - 1),
                    )
                nc.vector.tensor_copy(entT[:, mt, :], p[:])
            for mt in range(MT_D):
                p = ps.tile([128, E], f32, tag="mm")
                nc.tensor.matmul(
                    p[:],
                    embw[:, mt * 128 : (mt + 1) * 128],
                    ohlab[:],
                    start=True,
                    stop=True,
                )
                nc.vector.tensor_copy(entT[:, MT_D + mt, :], p[:])

            # ---- ffnn chains; head first so tail weights can still stream ----
            h2T = {}

            def ffnn(side):
                h1T = act.tile(
                    [128, KT_H1, E], f32r, tag=f"h1T{side}", name=f"h1T{side}"
                )
                for mt in range(MT_H1):
                    p = ps.tile([128, E], f32, tag="mm")
                    for kt in range(KT_MLP):
                        nc.tensor.matmul(
                            p[:],
                            w1[side][:, kt, mt * 128 : (mt + 1) * 128],
                            entT[:, kt, :],
                            start=(kt == 0),
                            stop=(kt == KT_MLP - 1),
                        )
                    nc.scalar.activation(
                        h1T[:, mt, :],
                        p[:],
                        mybir.ActivationFunctionType.Relu,
                        bias=b1[side][:, mt : mt + 1],
                    )
                h2T[side] = act.tile(
                    [128, KT_H2, E], f32r, tag=f"h2T{side}", name=f"h2T{side}"
                )
                for mt in range(MT_H2):
                    p = ps.tile([128, E], f32, tag="mm")
                    for kt in range(KT_H1):
                        nc.tensor.matmul(
                            p[:],
                            w2[side][:, kt, mt * 128 : (mt + 1) * 128],
                            h1T[:, kt, :],
                            start=(kt == 0),
                            stop=(kt == KT_H1 - 1),
                        )
                    nc.scalar.activation(
                        h2T[side][:, mt, :],
                        p[:],
                        mybir.ActivationFunctionType.Relu,
                        bias=b2[side][:, mt : mt + 1],
                    )

            ffnn("h")

            # ---- N_o^T and lin_h (depend only on the head chain) ----
            nT = []
            for o in range(OUT):
                nTo = act.tile([128, KT_H2, E], f32r, tag=f"nT{o}", name=f"nT{o}")
                for mt in range(MT_H2):
                    p = ps.tile([128, E], f32, tag="mm")
                    for kt in range(KT_H2):
                        nc.tensor.matmul(
                            p[:],
                            wb[o][:, kt, mt * 128 : (mt + 1) * 128],
                            h2T["h"][:, kt, :],
                            start=(kt == 0),
                            stop=(kt == KT_H2 - 1),
                        )
                    nc.vector.tensor_copy(nTo[:, mt, :], p[:])
                nT.append(nTo)

            linh = []
            for o in range(OUT):
                lh = act.tile([1, E], f32r, tag=f"linh{o}", name=f"linh{o}")
                p = ps1.tile([1, E], f32, tag="lin")
                for kt in range(KT_H2):
                    nc.tensor.matmul(
                        p[:],
                        wlin[:, kt, o : o + 1],
                        h2T["h"][:, kt, :],
                        start=(kt == 0),
                        stop=(kt == KT_H2 - 1),
                    )
                nc.vector.tensor_copy(lh[:], p[:])
                linh.append(lh)

            ffnn("t")

            lint = []
            for o in range(OUT):
                lt = act.tile([1, E], f32r, tag=f"lint{o}", name=f"lint{o}")
                p = ps1.tile([1, E], f32, tag="lin")
                for kt in range(KT_H2):
                    nc.tensor.matmul(
                        p[:],
                        wlin[:, KT_H2 + kt, o : o + 1],
                        h2T["t"][:, kt, :],
                        start=(kt == 0),
                        stop=(kt == KT_H2 - 1),
                    )
                # + b_lin[o] folded in via bias
                nc.scalar.activation(
                    lt[:],
                    p[:],
                    mybir.ActivationFunctionType.Identity,
                    bias=blin[:, o : o + 1],
                )
                lint.append(lt)

            # ---- table slab [128, 2*E, OUT]: partition p holds e1=p rows
            #      (elems 0:256) and e1=128+p rows (elems 256:512) ----
            slab = act.tile([128, 2 * E, OUT], f32, tag="slab")
            for o in range(OUT):
                for mt in range(MT_E):
                    p = ps.tile([128, E], f32, tag="mm")
                    for kt in range(KT_H2):
                        nc.tensor.matmul(
                            p[:],
                            nT[o][:, kt, mt * 128 : (mt + 1) * 128],
                            h2T["t"][:, kt, :],
                            start=(kt == 0),
                            stop=False,
                        )
                    nc.tensor.matmul(
                        p[:],
                        linh[o][:, mt * 128 : (mt + 1) * 128],
                        ones[:],
                        start=False,
                        stop=False,
                    )
                    nc.tensor.matmul(
                        p[:],
                        ones[:, 0:128],
                        lint[o][:],
                        start=False,
                        stop=True,
                    )
                    nc.vector.tensor_copy(slab[:, mt * E : (mt + 1) * E, o], p[:])

            # ---- gather + output ----
            # Bounce the slab through an SBUF->SBUF DMA: a gather whose input
            # was last written by DVE gets a ~61us GPSIMD drain scheduled in
            # front of it; a DMA-written input does not.
            slab2 = act.tile([128, 2 * E, OUT], f32, tag="slab2")
            nc.sync.dma_start(slab2[:], slab[:])
            gout = act.tile([128, ni, OUT], f32, tag="gout")
            nc.gpsimd.ap_gather(
                gout[:], slab2[:], idx[:], channels=128, num_elems=2 * E, d=OUT,
                num_idxs=ni,
            )
            nc.sync.dma_start(d_gout.ap(), gout[:])

    nc.compile()
    _cache[ni] = nc
    return nc


def _pack(w, kt):
    """[kt*128, n] row-major -> [128, kt*n] partition-packed."""
    n = w.shape[1]
    return np.ascontiguousarray(
        w.reshape(kt, 128, n).transpose(1, 0, 2).reshape(128, kt * n)
    )


def _prep_host(inputs):
    """Host-side index preprocessing -> per-core in_maps + assembly info."""
    hs = np.asarray(inputs["hidden_states"], dtype=np.float32)
    start = np.asarray(inputs["entity_start"]).astype(np.int64)
    end = np.asarray(inputs["entity_end"]).astype(np.int64)
    label = np.asarray(inputs["entity_label"]).astype(np.int64)
    head_idx = np.asarray(inputs["head_idx"]).astype(np.int64)
    tail_idx = np.asarray(inputs["tail_idx"]).astype(np.int64)

    t = np.arange(T)
    mask = (
        (t[None, None, :] >= start[:, :, None]) & (t[None, None, :] < end[:, :, None])
    ).astype(np.float32)  # [B,E,T]
    counts = np.maximum(mask.sum(-1, keepdims=True), 1.0)
    masknT = (mask / counts).transpose(0, 2, 1)  # [B,T,E]

    ohlab = np.zeros((B, NL, E), np.float32)
    for b in range(B):
        ohlab[b, label[b], np.arange(E)] = 1.0

    def f32(x):
        return np.ascontiguousarray(np.asarray(x, dtype=np.float32))

    w_bil = f32(inputs["W_bil"])
    shared = {
        "embw": f32(inputs["entity_emb_w"]),
        "Wh1": _pack(f32(inputs["Wh1"]), KT_MLP),
        "Wt1": _pack(f32(inputs["Wt1"]), KT_MLP),
        "Wh2": _pack(f32(inputs["Wh2"]), KT_H1),
        "Wt2": _pack(f32(inputs["Wt2"]), KT_H1),
        "Wbil0": _pack(w_bil[0], KT_H2),
        "Wbil1": _pack(w_bil[1], KT_H2),
        "Wlin": _pack(f32(inputs["W_lin"]), 2 * KT_H2),
        "blin": f32(inputs["b_lin"]).reshape(1, OUT),
        "ones": np.ones((1, E), np.float32),
        "bh1t": np.ascontiguousarray(f32(inputs["bh1"]).reshape(MT_H1, 128).T),
        "bt1t": np.ascontiguousarray(f32(inputs["bt1"]).reshape(MT_H1, 128).T),
        "bh2t": np.ascontiguousarray(f32(inputs["bh2"]).reshape(MT_H2, 128).T),
        "bt2t": np.ascontiguousarray(f32(inputs["bt2"]).reshape(MT_H2, 128).T),
    }

    # --- pair bucketing per core ---
    q = P // 4  # pairs per core
    cores = []
    ni_needed = 0
    for i in range(N_CORES):
        b, quarter = divmod(i, 4)
        sl = slice(quarter * q, (quarter + 1) * q)
        e1 = head_idx[b, sl]
        e2 = tail_idx[b, sl]
        part = e1 % 128  # target partition (= gpsimd channel)
        gcore = part // 16  # gpsimd core 0..7
        elem = e2 + 256 * (e1 // 128)  # index into per-partition table row pair
        order = np.argsort(gcore, kind="stable")
        counts_g = np.bincount(gcore, minlength=8)
        ni_needed = max(ni_needed, int(counts_g.max()))
        cores.append((b, sl, part, order, counts_g, elem))

    ni = -(-ni_needed // 16) * 16  # round up to multiple of 16

    in_maps = []
    assembly = []
    for i in range(N_CORES):
        b, sl, part, order, counts_g, elem = cores[i]
        elem_sorted = elem[order]
        gcore_sorted = (part // 16)[order]
        starts = np.zeros(8, np.int64)
        starts[1:] = np.cumsum(counts_g)[:-1]
        slot = np.arange(len(order)) - starts[gcore_sorted]  # slot within bucket
        idx_arr = np.zeros((128, ni // 16), np.int16)
        for j in range(8):
            lj = elem_sorted[gcore_sorted == j].astype(np.int16)
            pad = np.zeros(ni, np.int16)
            pad[: len(lj)] = lj
            idx_arr[16 * j : 16 * (j + 1)] = einops.rearrange(pad, "(s p) -> p s", p=16)
        m = dict(shared)
        m["hs"] = _pack(hs[b], KT_T)
        m["masknT"] = _pack(masknT[b], KT_T)
        m["ohlab"] = np.ascontiguousarray(ohlab[b])
        m["idx"] = idx_arr
        in_maps.append(m)
        # assembly: out[b, sl][order] = gout[part_sorted, slot, :]
        assembly.append((b, sl, part[order], slot, order))

    return in_maps, assembly, ni


def kernel(**inputs) -> np.ndarray:
    in_maps, assembly, ni = _prep_host(inputs)
    nc = _build(ni)
    res = run_bass_kernel_spmd(nc, in_maps, list(range(N_CORES)))
    out = np.zeros((B, P, OUT), np.float32)
    for i in range(N_CORES):
        b, sl, part_sorted, slot, order = assembly[i]
        gathered = res.results[i]["gout"][part_sorted, slot, :]  # [q, OUT]
        block = np.empty_like(gathered)
        block[order] = gathered
        out[b, sl] = block
    return out


# revision 9
# speedup vs baseline: 1.0564x; 1.0321x over previous
"""Trainium2 Bass kernel for nn_CellDecoder (span-pool + ffnn + biaffine pairs).

Strategy: head_idx/tail_idx only reference E=256 entities, so instead of
computing the biaffine per pair (P=65536), each core builds the full E x E
biaffine logit table for its batch (small matmuls, fp32r) and the per-pair
work becomes a pure table lookup done with the GPSIMD ap_gather custom op.

Sharding: cores 0-3 handle batch 0, cores 4-7 batch 1. Each core replicates
its batch's table build and gathers its quarter of that batch's pairs
(bucketed host-side by e1%128//16 onto the 8 GPSIMD cores).

Perf notes:
- All matmul operands are float32r (TF32-like, 1 cyc/row at n>=256 vs 4 for
  fp32); inputs are declared float32r in DRAM so plain HWDGE DMAs feed the
  PE without a cast pass and the GPSIMD queue stays empty.
- The ap_gather ucode library is loaded explicitly at kernel start so the
  ModifyPoolConfig overlaps the weight stream instead of serializing before
  the gather (it costs ~60us when auto-inserted late).
- Big tensors are host-packed to dense [128, cols] so every DMA descriptor
  is a large contiguous run.
- DMA issue order matches compute order: pooling operands, head-ffnn
  weights, biaffine weights, tail-ffnn weights.
"""

import os

os.environ.setdefault("JAX_PLATFORMS", "axon,cpu")

import numpy as np
import einops

import concourse.bass as bass
import concourse.tile as tile
from concourse import bacc, mybir, library_config
from concourse.bass_utils import run_bass_kernel_spmd

dt = mybir.dt

B, T, D, E, P = 2, 512, 768, 256, 65536
MLP = 2 * D  # 1536
H1, H2 = MLP // 2, MLP // 4  # 768, 384
NL = 5
OUT = 2
N_CORES = 8

KT_MLP = MLP // 128  # 12
KT_H1 = H1 // 128  # 6
KT_H2 = H2 // 128  # 3
KT_T = T // 128  # 4
MT_D = D // 128  # 6
MT_H1 = H1 // 128  # 6
MT_H2 = H2 // 128  # 3
MT_E = E // 128  # 2

_cache: dict = {}


def _build(ni: int):
    """Build + compile the SPMD program; ni = padded per-gpsimd-core index count."""
    if ni in _cache:
        return _cache[ni]

    nc = bacc.Bacc("TRN2", target_bir_lowering=False, debug=False, num_devices=N_CORES)

    f32, f32r, i16 = dt.float32, dt.float32r, dt.int16

    # [128, cols] host-packed operand tensors (f32r bits == f32 bits)
    d_hs = nc.dram_tensor("hs", [128, KT_T * D], f32r, kind="ExternalInput")
    d_maskn = nc.dram_tensor("masknT", [128, KT_T * E], f32r, kind="ExternalInput")
    d_ohlab = nc.dram_tensor("ohlab", [NL, E], f32r, kind="ExternalInput")
    d_embw = nc.dram_tensor("embw", [NL, D], f32r, kind="ExternalInput")
    d_wh1 = nc.dram_tensor("Wh1", [128, KT_MLP * H1], f32r, kind="ExternalInput")
    d_wt1 = nc.dram_tensor("Wt1", [128, KT_MLP * H1], f32r, kind="ExternalInput")
    d_wh2 = nc.dram_tensor("Wh2", [128, KT_H1 * H2], f32r, kind="ExternalInput")
    d_wt2 = nc.dram_tensor("Wt2", [128, KT_H1 * H2], f32r, kind="ExternalInput")
    d_bh1 = nc.dram_tensor("bh1t", [128, MT_H1], f32, kind="ExternalInput")
    d_bt1 = nc.dram_tensor("bt1t", [128, MT_H1], f32, kind="ExternalInput")
    d_bh2 = nc.dram_tensor("bh2t", [128, MT_H2], f32, kind="ExternalInput")
    d_bt2 = nc.dram_tensor("bt2t", [128, MT_H2], f32, kind="ExternalInput")
    d_wb0 = nc.dram_tensor("Wbil0", [128, KT_H2 * H2], f32r, kind="ExternalInput")
    d_wb1 = nc.dram_tensor("Wbil1", [128, KT_H2 * H2], f32r, kind="ExternalInput")
    d_wlin = nc.dram_tensor("Wlin", [128, 2 * KT_H2 * OUT], f32r, kind="ExternalInput")
    d_blin = nc.dram_tensor("blin", [1, OUT], f32, kind="ExternalInput")
    d_ones = nc.dram_tensor("ones", [1, E], f32r, kind="ExternalInput")
    d_idx = nc.dram_tensor("idx", [128, ni // 16], i16, kind="ExternalInput")
    d_gout = nc.dram_tensor("gout", [128, ni, OUT], f32, kind="ExternalOutput")

    with tile.TileContext(nc) as tc:
        with (
            tc.tile_pool(name="wbig", bufs=1) as wbig,
            tc.tile_pool(name="wsml", bufs=1) as wsml,
            tc.tile_pool(name="act", bufs=1) as act,
            tc.tile_pool(name="ps", bufs=4, space="PSUM") as ps,
            tc.tile_pool(name="ps1", bufs=2, space="PSUM") as ps1,
        ):
            # ap_gather ucode load up front, overlapping the DMA stream
            nc.gpsimd.load_library(library_config.ap_gather)

            def load(pool, name, dram, shape, dtype=f32r, engine=None):
                t = pool.tile(shape, dtype, tag=name, name=name)
                src = dram.ap()
                if len(shape) == 3:
                    src = src.rearrange("p (kt n) -> p kt n", kt=shape[1])
                (engine or nc.sync).dma_start(t[:], src)
                return t

            # smalls on the scalar HWDGE ring so the sync ring is all bulk
            idx = load(wsml, "idx", d_idx, [128, ni // 16], i16, nc.scalar)
            blin = load(wsml, "blin", d_blin, [1, OUT], f32, nc.scalar)
            ones = load(wsml, "ones", d_ones, [1, E], f32r, nc.scalar)
            b1 = {
                "h": load(wsml, "b1h", d_bh1, [128, MT_H1], f32, nc.scalar),
                "t": load(wsml, "b1t", d_bt1, [128, MT_H1], f32, nc.scalar),
            }
            b2 = {
                "h": load(wsml, "b2h", d_bh2, [128, MT_H2], f32, nc.scalar),
                "t": load(wsml, "b2t", d_bt2, [128, MT_H2], f32, nc.scalar),
            }
            ohlab = load(wsml, "ohlab", d_ohlab, [NL, E], f32r, nc.scalar)
            embw = load(wsml, "embw", d_embw, [NL, D], f32r, nc.scalar)

            # bulk stream in compute order
            hs = load(wbig, "hs", d_hs, [128, KT_T, D])
            maskn = load(wsml, "maskn", d_maskn, [128, KT_T, E])
            w1 = {"h": load(wbig, "w1h", d_wh1, [128, KT_MLP, H1])}
            w2 = {"h": load(wbig, "w2h", d_wh2, [128, KT_H1, H2])}
            wb = [
                load(wsml, "wb0", d_wb0, [128, KT_H2, H2]),
                load(wsml, "wb1", d_wb1, [128, KT_H2, H2]),
            ]
            wlin = load(wsml, "wlin", d_wlin, [128, 2 * KT_H2, OUT])
            w1["t"] = load(wbig, "w1t", d_wt1, [128, KT_MLP, H1])
            w2["t"] = load(wbig, "w2t", d_wt2, [128, KT_H1, H2])

            # ---- ent_repr^T = [pooled^T ; emb^T]  [128, 12, E] (f32r) ----
            entT = act.tile([128, KT_MLP, E], f32r, tag="entT")
            for mt in range(MT_D):
                p = ps.tile([128, E], f32, tag="mm")
                for kt in range(KT_T):
                    nc.tensor.matmul(
                        p[:],
                        hs[:, kt, mt * 128 : (mt + 1) * 128],
                        maskn[:, kt, :],
                        start=(kt == 0),
                        stop=(kt == KT_T - 1),
                    )
                nc.vector.tensor_copy(entT[:, mt, :], p[:])
            for mt in range(MT_D):
                p = ps.tile([128, E], f32, tag="mm")
                nc.tensor.matmul(
                    p[:],
                    embw[:, mt * 128 : (mt + 1) * 128],
                    ohlab[:],
                    start=True,
                    stop=True,
                )
                nc.vector.tensor_copy(entT[:, MT_D + mt, :], p[:])

            # ---- ffnn chains; head first so tail weights can still stream ----
            h2T = {}

            def ffnn(side):
                h1T = act.tile(
                    [128, KT_H1, E], f32r, tag=f"h1T{side}", name=f"h1T{side}"
                )
                for mt in range(MT_H1):
                    p = ps.tile([128, E], f32, tag="mm")
                    for kt in range(KT_MLP):
                        nc.tensor.matmul(
                            p[:],
                            w1[side][:, kt, mt * 128 : (mt + 1) * 128],
                            entT[:, kt, :],
                            start=(kt == 0),
                            stop=(kt == KT_MLP - 1),
                        )
                    nc.scalar.activation(
                        h1T[:, mt, :],
                        p[:],
                        mybir.ActivationFunctionType.Relu,
                        bias=b1[side][:, mt : mt + 1],
                    )
                h2T[side] = act.tile(
                    [128, KT_H2, E], f32r, tag=f"h2T{side}", name=f"h2T{side}"
                )
                for mt in range(MT_H2):
                    p = ps.tile([128, E], f32, tag="mm")
                    for kt in range(KT_H1):
                        nc.tensor.matmul(
                            p[:],
                            w2[side][:, kt, mt * 128 : (mt + 1) * 128],
                            h1T[:, kt, :],
                            start=(kt == 0),
                            stop=(kt == KT_H1 - 1),
                        )
                    nc.scalar.activation(
                        h2T[side][:, mt, :],
                        p[:],
                        mybir.ActivationFunctionType.Relu,
                        bias=b2[side][:, mt : mt + 1],
                    )

            ffnn("h")

            # ---- N_o^T and lin_h (depend only on the head chain) ----
            nT = []
            for o in range(OUT):
                nTo = act.tile([128, KT_H2, E], f32r, tag=f"nT{o}", name=f"nT{o}")
                for mt in range(MT_H2):
                    p = ps.tile([128, E], f32, tag="mm")
                    for kt in range(KT_H2):
                        nc.tensor.matmul(
                            p[:],
                            wb[o][:, kt, mt * 128 : (mt + 1) * 128],
                            h2T["h"][:, kt, :],
                            start=(kt == 0),
                            stop=(kt == KT_H2 - 1),
                        )
                    nc.vector.tensor_copy(nTo[:, mt, :], p[:])
                nT.append(nTo)

            linh = []
            for o in range(OUT):
                lh = act.tile([1, E], f32r, tag=f"linh{o}", name=f"linh{o}")
                p = ps1.tile([1, E], f32, tag="lin")
                for kt in range(KT_H2):
                    nc.tensor.matmul(
                        p[:],
                        wlin[:, kt, o : o + 1],
                        h2T["h"][:, kt, :],
                        start=(kt == 0),
                        stop=(kt == KT_H2 - 1),
                    )
                nc.vector.tensor_copy(lh[:], p[:])
                linh.append(lh)

            ffnn("t")

            lint = []
            for o in range(OUT):
                lt = act.tile([1, E], f32r, tag=f"lint{o}", name=f"lint{o}")
                p = ps1.tile([1, E], f32, tag="lin")
                for kt in range(KT_H2):
                    nc.tensor.matmul(
                        p[:],
                        wlin[:, KT_H2 + kt, o : o + 1],
                        h2T["t"][:, kt, :],
                        start=(kt == 0),
                        stop=(kt == KT_H2 - 1),
                    )
                # + b_lin[o] folded in via bias
                nc.scalar.activation(
                    lt[:],
                    p[:],
                    mybir.ActivationFunctionType.Identity,
                    bias=blin[:, o : o + 1],
                )
                lint.append(lt)

            # ---- table slab [128, 2*E, OUT]: partition p holds e1=p rows
            #      (elems 0:256) and e1=128+p rows (elems 256:512) ----
            slab = act.tile([128, 2 * E, OUT], f32, tag="slab")
            for o in range(OUT):
                for mt in range(MT_E):
                    p = ps.tile([128, E], f32, tag="mm")
                    for kt in range(KT_H2):
                        nc.tensor.matmul(
                            p[:],
                            nT[o][:, kt, mt * 128 : (mt + 1) * 128],
                            h2T["t"][:, kt, :],
                            start=(kt == 0),
                            stop=False,
                        )
                    nc.tensor.matmul(
                        p[:],
                        linh[o][:, mt * 128 : (mt + 1) * 128],
                        ones[:],
                        start=False,
                        stop=False,
                    )
                    nc.tensor.matmul(
                        p[:],
                        ones[:, 0:128],
                        lint[o][:],
                        start=False,
                        stop=True,
                    )
                    nc.vector.tensor_copy(slab[:, mt * E : (mt + 1) * E, o], p[:])

            # ---- gather + output ----
            gout = act.tile([128, ni, OUT], f32, tag="gout")
            nc.gpsimd.ap_gather(
                gout[:], slab[:], idx[:], channels=128, num_elems=2 * E, d=OUT,
                num_idxs=ni,
            )
            nc.sync.dma_start(d_gout.ap(), gout[:])

    nc.compile()
    _cache[ni] = nc
    return nc


def _pack(w, kt):
    """[kt*128, n] row-major -> [128, kt*n] partition-packed."""
    n = w.shape[1]
    return np.ascontiguousarray(
        w.reshape(kt, 128, n).transpose(1, 0, 2).reshape(128, kt * n)
    )


def _prep_host(inputs):
    """Host-side index preprocessing -> per-core in_maps + assembly info."""
    hs = np.asarray(inputs["hidden_states"], dtype=np.float32)
    start = np.asarray(inputs["entity_start"]).astype(np.int64)
    end = np.asarray(inputs["entity_end"]).astype(np.int64)
    label = np.asarray(inputs["entity_label"]).astype(np.int64)
    head_idx = np.asarray(inputs["head_idx"]).astype(np.int64)
    tail_idx = np.asarray(inputs["tail_idx"]).astype(np.int64)

    t = np.arange(T)
    mask = (
        (t[None, None, :] >= start[:, :, None]) & (t[None, None, :] < end[:, :, None])
    ).astype(np.float32)  # [B,E,T]
    counts = np.maximum(mask.sum(-1, keepdims=True), 1.0)
    masknT = (mask / counts).transpose(0, 2, 1)  # [B,T,E]

    ohlab = np.zeros((B, NL, E), np.float32)
    for b in range(B):
        ohlab[b, label[b], np.arange(E)] = 1.0

    def f32(x):
        return np.ascontiguousarray(np.asarray(x, dtype=np.float32))

    w_bil = f32(inputs["W_bil"])
    shared = {
        "embw": f32(inputs["entity_emb_w"]),
        "Wh1": _pack(f32(inputs["Wh1"]), KT_MLP),
        "Wt1": _pack(f32(inputs["Wt1"]), KT_MLP),
        "Wh2": _pack(f32(inputs["Wh2"]), KT_H1),
        "Wt2": _pack(f32(inputs["Wt2"]), KT_H1),
        "Wbil0": _pack(w_bil[0], KT_H2),
        "Wbil1": _pack(w_bil[1], KT_H2),
        "Wlin": _pack(f32(inputs["W_lin"]), 2 * KT_H2),
        "blin": f32(inputs["b_lin"]).reshape(1, OUT),
        "ones": np.ones((1, E), np.float32),
        "bh1t": np.ascontiguousarray(f32(inputs["bh1"]).reshape(MT_H1, 128).T),
        "bt1t": np.ascontiguousarray(f32(inputs["bt1"]).reshape(MT_H1, 128).T),
        "bh2t": np.ascontiguousarray(f32(inputs["bh2"]).reshape(MT_H2, 128).T),
        "bt2t": np.ascontiguousarray(f32(inputs["bt2"]).reshape(MT_H2, 128).T),
    }

    # --- pair bucketing per core ---
    q = P // 4  # pairs per core
    cores = []
    ni_needed = 0
    for i in range(N_CORES):
        b, quarter = divmod(i, 4)
        sl = slice(quarter * q, (quarter + 1) * q)
        e1 = head_idx[b, sl]
        e2 = tail_idx[b, sl]
        part = e1 % 128  # target partition (= gpsimd channel)
        gcore = part // 16  # gpsimd core 0..7
        elem = e2 + 256 * (e1 // 128)  # index into per-partition table row pair
        order = np.argsort(gcore, kind="stable")
        counts_g = np.bincount(gcore, minlength=8)
        ni_needed = max(ni_needed, int(counts_g.max()))
        cores.append((b, sl, part, order, counts_g, elem))

    ni = -(-ni_needed // 16) * 16  # round up to multiple of 16

    in_maps = []
    assembly = []
    for i in range(N_CORES):
        b, sl, part, order, counts_g, elem = cores[i]
        elem_sorted = elem[order]
        gcore_sorted = (part // 16)[order]
        starts = np.zeros(8, np.int64)
        starts[1:] = np.cumsum(counts_g)[:-1]
        slot = np.arange(len(order)) - starts[gcore_sorted]  # slot within bucket
        idx_arr = np.zeros((128, ni // 16), np.int16)
        for j in range(8):
            lj = elem_sorted[gcore_sorted == j].astype(np.int16)
            pad = np.zeros(ni, np.int16)
            pad[: len(lj)] = lj
            idx_arr[16 * j : 16 * (j + 1)] = einops.rearrange(pad, "(s p) -> p s", p=16)
        m = dict(shared)
        m["hs"] = _pack(hs[b], KT_T)
        m["masknT"] = _pack(masknT[b], KT_T)
        m["ohlab"] = np.ascontiguousarray(ohlab[b])
        m["idx"] = idx_arr
        in_maps.append(m)
        # assembly: out[b, sl][order] = gout[part_sorted, slot, :]
        assembly.append((b, sl, part[order], slot, order))

    return in_maps, assembly, ni


def kernel(**inputs) -> np.ndarray:
    in_maps, assembly, ni = _prep_host(inputs)
    nc = _build(ni)
    res = run_bass_kernel_spmd(nc, in_maps, list(range(N_CORES)))
    out = np.zeros((B, P, OUT), np.float32)
    for i in range(N_CORES):
        b, sl, part_sorted, slot, order = assembly[i]
        gathered = res.results[i]["gout"][part_sorted, slot, :]  # [q, OUT]
        block = np.empty_like(gathered)
        block[order] = gathered
        out[b, sl] = block
    return out
